# revision 1
# baseline (speedup 1.0000x reference)
"""LDPC encoder kernel for Trainium2 (8 NeuronCores, batch-sharded).

Computes out = 1 - 2*((m @ G^T) mod 2)  (BPSK-mapped LDPC codeword).

  m: [16384, 1200] int32 (0/1)   G: [2400, 1200] float32 (0/1)
  out: [16384, 2400] float32 (+-1)

Strategy:
  - Shard the batch over 8 cores (2048 rows each); G replicated.
  - G is systematic (G[:1200] == I), so out[:, :1200] = 1 - 2*m is a pure
    elementwise map; only the 1200 parity columns need a matmul.
  - Matmul in bf16 (values 0/1/2 are exact; PSUM accumulates fp32 exactly).
    Host feeds m transposed ([K,B] layout) so the stationary operand needs
    no on-device transpose, plus G^T scaled by 2 with an extra all-ones/2
    bias row so PSUM holds 2*d + 2. Then a single DVE op per tile:
        out = (psum mod 4) - 1  ->  {+1 even d, -1 odd d}
  - Output written as bf16 (+-1 exact), cast to f32 on host.
"""

import numpy as np
import ml_dtypes

BF16 = ml_dtypes.bfloat16

B_FULL = 16384
K_MSG = 1200
N_BITS = 2400
N_CORES = 8
B_LOC = B_FULL // N_CORES  # 2048
K_PAD = 1280  # 10 k-tiles of 128; row 1200 is the +2 bias row
P = 128

_CACHE: dict = {}
# fp8 DoubleRow matmul (2 contraction rows per PE cell): compiles and is
# exact in CoreSim, but the generated NEFF hit NRT_EXEC_UNIT_UNRECOVERABLE
# on hardware — keep the proven bf16 path.
USE_DR = False


def _mm_np_dtype():
    if not USE_DR:
        return BF16
    import concourse.mybir as mybir
    return mybir.dt.np(mybir.dt.float8e4)


def _build(bl, k_msg, k_pad, n_par, n_bits, base_col, with_identity,
           use_dr=False):
    """Build + compile the per-core Bass program.

    bl: local batch rows; n_par: matmul output columns; base_col: where the
    matmul columns land in the output; with_identity: also emit
    out[:, :k_msg] = 1-2*m from a natural-layout copy of m.
    """
    import concourse.bacc as bacc
    import concourse.mybir as mybir
    import concourse.tile as tile

    bf16 = mybir.dt.bfloat16
    f32 = mybir.dt.float32
    i32 = mybir.dt.int32
    Alu = mybir.AluOpType
    Act = mybir.ActivationFunctionType

    nc = bacc.Bacc("TRN2", target_bir_lowering=False, debug=False,
                   num_devices=N_CORES)

    fp8 = mybir.dt.float8e4
    mm_dt = fp8 if use_dr else bf16
    mT = nc.dram_tensor("mT", [k_pad, bl], mm_dt, kind="ExternalInput")
    gT = nc.dram_tensor("GT2", [k_pad, n_par], mm_dt, kind="ExternalInput")
    out = nc.dram_tensor("out", [bl, n_bits], bf16, kind="ExternalOutput")
    mnat = None
    if with_identity:
        mnat = nc.dram_tensor("mnat", [bl, k_msg], bf16, kind="ExternalInput")

    k_step = 2 * P if use_dr else P
    kt_n = k_pad // k_step
    nb = bl // P
    chunks = []
    n0 = 0
    while n0 < n_par:
        w = min(512, n_par - n0)
        chunks.append((n0, w))
        n0 += w

    with tile.TileContext(nc) as tc:
        with (
            tc.tile_pool(name="const", bufs=1) as cpool,
            tc.tile_pool(name="mn", bufs=3) as mnpool,
            tc.tile_pool(name="po", bufs=6) as popool,
            tc.tile_pool(name="io", bufs=3) as iopool,
            tc.tile_pool(name="ps", bufs=6, space="PSUM") as pspool,
        ):
            gts, mts = [], []
            for t in range(kt_n):
                ks = slice(t * k_step, (t + 1) * k_step)
                if use_dr:
                    # [2*P, X] DRAM rows -> [P, 2, X] SBUF (k = t*256 + i*128 + p)
                    gt_t = cpool.tile([P, 2, n_par], mm_dt, tag=f"gt{t}")
                    nc.sync.dma_start(
                        out=gt_t[:],
                        in_=gT[ks, :].rearrange("(i p) c -> p i c", i=2))
                    mt_t = cpool.tile([P, 2, bl], mm_dt, tag=f"mt{t}")
                    nc.sync.dma_start(
                        out=mt_t[:],
                        in_=mT[ks, :].rearrange("(i p) c -> p i c", i=2))
                else:
                    gt_t = cpool.tile([P, n_par], mm_dt, tag=f"gt{t}")
                    nc.sync.dma_start(out=gt_t[:], in_=gT[ks, :])
                    mt_t = cpool.tile([P, bl], mm_dt, tag=f"mt{t}")
                    nc.sync.dma_start(out=mt_t[:], in_=mT[ks, :])
                gts.append(gt_t)
                mts.append(mt_t)

            for b in range(nb):
                bs = slice(b * P, (b + 1) * P)
                psts = [pspool.tile([P, 512], f32, tag="ps", name=f"ps{b}_{ci}")
                        for ci in range(len(chunks))]
                for t in range(kt_n):
                    for ci, (n0, w) in enumerate(chunks):
                        if use_dr:
                            nc.tensor.matmul(
                                psts[ci][:, :w],
                                mts[t][:, :, bs],
                                gts[t][:, :, n0:n0 + w],
                                start=(t == 0),
                                stop=(t == kt_n - 1),
                                perf_mode=mybir.MatmulPerfMode.DoubleRow,
                            )
                        else:
                            nc.tensor.matmul(
                                psts[ci][:, :w],
                                mts[t][:, bs],
                                gts[t][:, n0:n0 + w],
                                start=(t == 0),
                                stop=(t == kt_n - 1),
                            )
                for ci, (n0, w) in enumerate(chunks):
                    # parity -> BPSK: p = int(d) & 1 ; out = -2p + 1
                    it = popool.tile([P, 512], i32, tag="pi",
                                     name=f"pi{b}_{ci}")
                    nc.vector.tensor_copy(it[:, :w], psts[ci][:, :w])
                    pt = popool.tile([P, 512], i32, tag="pp",
                                     name=f"pp{b}_{ci}")
                    nc.vector.tensor_scalar(
                        pt[:, :w], it[:, :w], 1, None, op0=Alu.bitwise_and,
                    )
                    ot = popool.tile([P, 512], bf16, tag="po",
                                     name=f"po{b}_{ci}")
                    nc.vector.tensor_scalar(
                        ot[:, :w], pt[:, :w], -2.0, 1.0,
                        op0=Alu.mult, op1=Alu.add,
                    )
                    nc.sync.dma_start(
                        out=out[bs, base_col + n0:base_col + n0 + w],
                        in_=ot[:, :w],
                    )
                if with_identity:
                    mn = mnpool.tile([P, k_msg], bf16, tag="mn")
                    nc.sync.dma_start(out=mn[:], in_=mnat[bs, :])
                    io = iopool.tile([P, k_msg], bf16, tag="io")
                    nc.vector.tensor_scalar(
                        io[:], mn[:], -2.0, 1.0, op0=Alu.mult, op1=Alu.add,
                    )
                    nc.sync.dma_start(out=out[bs, 0:k_msg], in_=io[:])

    nc.compile()
    return nc


def _get_nc(fast: bool):
    key = ("fast" if fast else "full", USE_DR)
    if key not in _CACHE:
        if fast:
            _CACHE[key] = _build(B_LOC, K_MSG, K_PAD, N_BITS - K_MSG, N_BITS,
                                 K_MSG, True, use_dr=USE_DR)
        else:
            _CACHE[key] = _build(B_LOC, K_MSG, K_PAD, N_BITS, N_BITS, 0, False,
                                 use_dr=USE_DR)
    return _CACHE[key]


def _prep_inputs(m, G, fast: bool):
    """Host-side marshaling: casts, transposes, padding, bias row."""
    mm_dt = _mm_np_dtype()
    m_mm = m.astype(mm_dt)
    if fast:
        g_rows = G[K_MSG:N_BITS]  # parity rows only
    else:
        g_rows = G
    n_par = g_rows.shape[0]
    gT2 = np.zeros((K_PAD, n_par), dtype=mm_dt)
    gT2[:K_MSG] = g_rows.T.astype(mm_dt)  # psum = d (count of set bits)

    in_maps = []
    for c in range(N_CORES):
        m_c = m_mm[c * B_LOC:(c + 1) * B_LOC]
        mT = np.zeros((K_PAD, B_LOC), dtype=mm_dt)
        mT[:K_MSG] = np.ascontiguousarray(m_c.T)
        im = {"mT": mT, "GT2": gT2}
        if fast:
            im["mnat"] = np.ascontiguousarray(
                m[c * B_LOC:(c + 1) * B_LOC].astype(BF16))
        in_maps.append(im)
    return in_maps


def _run(m, G, trace=False):
    from concourse.bass_utils import run_bass_kernel_spmd

    fast = bool(
        np.array_equal(G[:K_MSG], np.eye(K_MSG, dtype=G.dtype))
        and ((G == 0) | (G == 1)).all()
    )
    nc = _get_nc(fast)
    in_maps = _prep_inputs(m, G, fast)
    res = run_bass_kernel_spmd(
        nc, in_maps, core_ids=list(range(N_CORES)), trace=trace,
    )
    parts = [res.results[c]["out"] for c in range(N_CORES)]
    full = np.concatenate(parts, axis=0).astype(np.float32)
    return full, res


def kernel(m, G, snr=None):
    m = np.asarray(m)
    G = np.asarray(G)
    full, _ = _run(m, G, trace=False)
    return full



# revision 4
# speedup vs baseline: 92.2355x; 92.2355x over previous
"""LDPC encoder kernel for Trainium2 (8 NeuronCores, batch-sharded).

Computes out = 1 - 2*((m @ G^T) mod 2)  (BPSK-mapped LDPC codeword).

  m: [16384, 1200] int32 (0/1)   G: [2400, 1200] float32 (0/1)
  out: [16384, 2400] float32 (+-1)

Strategy:
  - Shard the batch over 8 cores (2048 rows each); G replicated.
  - G is systematic (G[:1200] == I), so out[:, :1200] = 1 - 2*m is a pure
    elementwise map; only the 1200 parity columns need a matmul.
  - Matmul in bf16 (values 0/1/2 are exact; PSUM accumulates fp32 exactly).
    Host feeds m transposed ([K,B] layout) so the stationary operand needs
    no on-device transpose, plus G^T scaled by 2 with an extra all-ones/2
    bias row so PSUM holds 2*d + 2. Then a single DVE op per tile:
        out = (psum mod 4) - 1  ->  {+1 even d, -1 odd d}
  - Output written as bf16 (+-1 exact), cast to f32 on host.
"""

import numpy as np
import ml_dtypes

BF16 = ml_dtypes.bfloat16

B_FULL = 16384
K_MSG = 1200
N_BITS = 2400
N_CORES = 8
B_LOC = B_FULL // N_CORES  # 2048
K_PAD = 1280  # 10 k-tiles of 128; row 1200 is the +2 bias row
P = 128

_CACHE: dict = {}
# fp8 DoubleRow matmul (2 contraction rows per PE cell): compiles and is
# exact in CoreSim, but the generated NEFF hit NRT_EXEC_UNIT_UNRECOVERABLE
# on hardware — keep the proven bf16 path.
USE_DR = False


def _mm_np_dtype():
    if not USE_DR:
        return BF16
    import concourse.mybir as mybir
    return mybir.dt.np(mybir.dt.float8e4)


def _build(bl, k_msg, k_pad, n_par, n_bits, base_col, with_identity,
           use_dr=False, repeat=1):
    """Build + compile the per-core Bass program.

    bl: local batch rows; n_par: matmul output columns; base_col: where the
    matmul columns land in the output; with_identity: also emit
    out[:, :k_msg] = 1-2*m from a natural-layout copy of m.
    repeat: re-execute the whole body N times (timing builds only; the
    outputs are rewritten identically each pass).
    """
    import concourse.bacc as bacc
    import concourse.mybir as mybir
    import concourse.tile as tile

    bf16 = mybir.dt.bfloat16
    f32 = mybir.dt.float32
    i32 = mybir.dt.int32
    Alu = mybir.AluOpType
    Act = mybir.ActivationFunctionType

    nc = bacc.Bacc("TRN2", target_bir_lowering=False, debug=False,
                   num_devices=N_CORES)

    fp8 = mybir.dt.float8e4
    mm_dt = fp8 if use_dr else bf16
    mT = nc.dram_tensor("mT", [k_pad, bl], mm_dt, kind="ExternalInput")
    gT = nc.dram_tensor("GT2", [k_pad, n_par], mm_dt, kind="ExternalInput")
    out = nc.dram_tensor("out", [bl, n_bits], bf16, kind="ExternalOutput")
    mnat = None
    if with_identity:
        mnat = nc.dram_tensor("mnat", [bl, k_msg], bf16, kind="ExternalInput")

    k_step = 2 * P if use_dr else P
    kt_n = k_pad // k_step
    nb = bl // P
    chunks = []
    n0 = 0
    while n0 < n_par:
        w = min(512, n_par - n0)
        chunks.append((n0, w))
        n0 += w

    with tile.TileContext(nc) as tc:
        with (
            tc.tile_pool(name="const", bufs=1) as cpool,
            tc.tile_pool(name="mn", bufs=3) as mnpool,
            tc.tile_pool(name="po", bufs=6) as popool,
            tc.tile_pool(name="io", bufs=3) as iopool,
            tc.tile_pool(name="ps", bufs=6, space="PSUM") as pspool,
        ):
          for rep in range(repeat):
            gts, mts = [], []
            for t in range(kt_n):
                ks = slice(t * k_step, (t + 1) * k_step)
                if use_dr:
                    # [2*P, X] DRAM rows -> [P, 2, X] SBUF (k = t*256 + i*128 + p)
                    gt_t = cpool.tile([P, 2, n_par], mm_dt, tag=f"gt{t}")
                    nc.sync.dma_start(
                        out=gt_t[:],
                        in_=gT[ks, :].rearrange("(i p) c -> p i c", i=2))
                    mt_t = cpool.tile([P, 2, bl], mm_dt, tag=f"mt{t}")
                    nc.sync.dma_start(
                        out=mt_t[:],
                        in_=mT[ks, :].rearrange("(i p) c -> p i c", i=2))
                else:
                    gt_t = cpool.tile([P, n_par], mm_dt, tag=f"gt{t}")
                    nc.sync.dma_start(out=gt_t[:], in_=gT[ks, :])
                    mt_t = cpool.tile([P, bl], mm_dt, tag=f"mt{t}")
                    nc.sync.dma_start(out=mt_t[:], in_=mT[ks, :])
                gts.append(gt_t)
                mts.append(mt_t)

            for b in range(nb):
                bs = slice(b * P, (b + 1) * P)
                psts = [pspool.tile([P, 512], f32, tag="ps", name=f"ps{rep}_{b}_{ci}")
                        for ci in range(len(chunks))]
                for t in range(kt_n):
                    for ci, (n0, w) in enumerate(chunks):
                        if use_dr:
                            nc.tensor.matmul(
                                psts[ci][:, :w],
                                mts[t][:, :, bs],
                                gts[t][:, :, n0:n0 + w],
                                start=(t == 0),
                                stop=(t == kt_n - 1),
                                perf_mode=mybir.MatmulPerfMode.DoubleRow,
                            )
                        else:
                            nc.tensor.matmul(
                                psts[ci][:, :w],
                                mts[t][:, bs],
                                gts[t][:, n0:n0 + w],
                                start=(t == 0),
                                stop=(t == kt_n - 1),
                            )
                for ci, (n0, w) in enumerate(chunks):
                    # parity -> BPSK: p = int(d) & 1 ; out = -2p + 1
                    it = popool.tile([P, 512], i32, tag="pi",
                                     name=f"pi{rep}_{b}_{ci}")
                    nc.vector.tensor_copy(it[:, :w], psts[ci][:, :w])
                    pt = popool.tile([P, 512], i32, tag="pp",
                                     name=f"pp{rep}_{b}_{ci}")
                    nc.vector.tensor_scalar(
                        pt[:, :w], it[:, :w], 1, None, op0=Alu.bitwise_and,
                    )
                    ot = popool.tile([P, 512], bf16, tag="po",
                                     name=f"po{rep}_{b}_{ci}")
                    nc.vector.tensor_scalar(
                        ot[:, :w], pt[:, :w], -2.0, 1.0,
                        op0=Alu.mult, op1=Alu.add,
                    )
                    nc.sync.dma_start(
                        out=out[bs, base_col + n0:base_col + n0 + w],
                        in_=ot[:, :w],
                    )
                if with_identity:
                    mn = mnpool.tile([P, k_msg], bf16, tag="mn")
                    nc.sync.dma_start(out=mn[:], in_=mnat[bs, :])
                    io = iopool.tile([P, k_msg], bf16, tag="io")
                    nc.vector.tensor_scalar(
                        io[:], mn[:], -2.0, 1.0, op0=Alu.mult, op1=Alu.add,
                    )
                    nc.sync.dma_start(out=out[bs, 0:k_msg], in_=io[:])

    nc.compile()
    return nc


def _get_nc(fast: bool):
    key = ("fast" if fast else "full", USE_DR)
    if key not in _CACHE:
        if fast:
            _CACHE[key] = _build(B_LOC, K_MSG, K_PAD, N_BITS - K_MSG, N_BITS,
                                 K_MSG, True, use_dr=USE_DR)
        else:
            _CACHE[key] = _build(B_LOC, K_MSG, K_PAD, N_BITS, N_BITS, 0, False,
                                 use_dr=USE_DR)
    return _CACHE[key]


def _prep_inputs(m, G, fast: bool):
    """Host-side marshaling: casts, transposes, padding, bias row."""
    mm_dt = _mm_np_dtype()
    m_mm = m.astype(mm_dt)
    if fast:
        g_rows = G[K_MSG:N_BITS]  # parity rows only
    else:
        g_rows = G
    n_par = g_rows.shape[0]
    gT2 = np.zeros((K_PAD, n_par), dtype=mm_dt)
    gT2[:K_MSG] = g_rows.T.astype(mm_dt)  # psum = d (count of set bits)

    in_maps = []
    for c in range(N_CORES):
        m_c = m_mm[c * B_LOC:(c + 1) * B_LOC]
        mT = np.zeros((K_PAD, B_LOC), dtype=mm_dt)
        mT[:K_MSG] = np.ascontiguousarray(m_c.T)
        im = {"mT": mT, "GT2": gT2}
        if fast:
            im["mnat"] = np.ascontiguousarray(
                m[c * B_LOC:(c + 1) * B_LOC].astype(BF16))
        in_maps.append(im)
    return in_maps


def _run(m, G, trace=False):
    from concourse.bass_utils import run_bass_kernel_spmd

    fast = bool(
        np.array_equal(G[:K_MSG], np.eye(K_MSG, dtype=G.dtype))
        and ((G == 0) | (G == 1)).all()
    )
    nc = _get_nc(fast)
    in_maps = _prep_inputs(m, G, fast)
    res = run_bass_kernel_spmd(
        nc, in_maps, core_ids=list(range(N_CORES)), trace=trace,
    )
    parts = [res.results[c]["out"] for c in range(N_CORES)]
    full = np.concatenate(parts, axis=0).astype(np.float32)
    return full, res


def kernel(m, G, snr=None):
    m = np.asarray(m)
    G = np.asarray(G)
    full, _ = _run(m, G, trace=False)
    return full



# revision 13
# speedup vs baseline: 305.4331x; 3.3114x over previous
"""LDPC encoder kernel for Trainium2 (8 NeuronCores, batch-sharded).

Computes out = 1 - 2*((m @ G^T) mod 2)  (BPSK-mapped LDPC codeword).

  m: [16384, 1200] int32 (0/1)   G: [2400, 1200] float32 (0/1)
  out: [16384, 2400] float32 (+-1)

Strategy:
  - Shard the batch over 8 cores (2048 rows each); G replicated.
  - G is systematic (G[:1200] == I), so out[:, :1200] = 1 - 2*m is a pure
    elementwise map; only the 1200 parity columns need a matmul.
  - Matmul in fp8e4 normal perf mode (values 0/1 exact; PSUM accumulates
    fp32 exactly; fp8 streams at the same PE rate as bf16 but halves the
    HBM input traffic). Host pre-tiles both operands so every DMA is a
    contiguous [128, X] transfer with >=512B partition lines:
      mb[b]  = [128(k-in-tile), 10*128] per 128-row batch block (stationary)
      gc[c]  = [128(k-in-tile), 10*w]   per output-column chunk  (moving)
  - Loop order: chunk-outer (c, then block, then k-tile). Chunk c of block
    b accumulates in one PSUM bank over 10 matmuls; the DVE drains it
    (f32->i32, &1, -2p+1 -> fp8) into a per-block [128, 2400] fp8 output
    tile while the PE runs the next block. The identity half is a DVE
    affine map from an fp8 copy of m, scheduled in pass 1 so its input DMA
    is off the critical path. One 300KB output DMA per block.
  - Output written as fp8e4 (+-1 exact), cast to f32 on host.
"""

import numpy as np
import ml_dtypes

BF16 = ml_dtypes.bfloat16

B_FULL = 16384
K_MSG = 1200
N_BITS = 2400
N_CORES = 8
B_LOC = B_FULL // N_CORES  # 2048
KT_N = 10                  # k tiles of 128 (1200 padded to 1280)
K_PAD = KT_N * 128
P = 128

_CACHE: dict = {}
# fp8 inputs/outputs (normal matmul perf mode — NOT DoubleRow, which hit
# NRT_EXEC_UNIT_UNRECOVERABLE on hardware in a previous session).
USE_FP8 = True


def _np_fp8():
    import concourse.mybir as mybir
    return mybir.dt.np(mybir.dt.float8e4)


def _mm_np_dtype():
    return _np_fp8() if USE_FP8 else BF16


def _chunks(n_par):
    # Even chunks <=512 (PSUM bank limit), sized so the N=w matmul stream
    # time exceeds the 128-col LDWEIGHTS time (~107ns) in every pass:
    # 1200 -> 3x400, 2400 -> 5x480.
    n_ch = -(-n_par // 512)
    w = -(-n_par // n_ch)
    out = []
    n0 = 0
    while n0 < n_par:
        out.append((n0, min(w, n_par - n0)))
        n0 += w
    return out


def _build(bl, k_msg, n_par, n_bits, base_col, with_identity, repeat=1):
    """Build + compile the per-core Bass program.

    bl: local batch rows; n_par: matmul output columns; base_col: where the
    matmul columns land in the output; with_identity: also emit
    out[:, :k_msg] = 1-2*m from an fp8 natural-layout copy of m.
    repeat: re-execute the whole body N times (timing builds only; the
    outputs are rewritten identically each pass).
    """
    import concourse.bacc as bacc
    import concourse.mybir as mybir
    import concourse.tile as tile

    f32 = mybir.dt.float32
    i32 = mybir.dt.int32
    Alu = mybir.AluOpType
    mm_dt = mybir.dt.float8e4 if USE_FP8 else mybir.dt.bfloat16
    out_dt = mybir.dt.float8e4 if USE_FP8 else mybir.dt.bfloat16

    nc = bacc.Bacc("TRN2", target_bir_lowering=False, debug=False,
                   num_devices=N_CORES)

    nb = bl // P
    chunks = _chunks(n_par)
    nch = len(chunks)

    # Host-pretiled inputs (see _prep_inputs for the exact layout).
    mb = nc.dram_tensor("mb", [nb, P, KT_N * P], mm_dt, kind="ExternalInput")
    gcs = [nc.dram_tensor(f"gc{ci}", [P, KT_N * w], mm_dt,
                          kind="ExternalInput")
           for ci, (n0, w) in enumerate(chunks)]
    out = nc.dram_tensor("out", [bl, n_bits], out_dt, kind="ExternalOutput")
    mnat = None
    if with_identity:
        mnat = nc.dram_tensor("mnat", [bl, k_msg], mm_dt,
                              kind="ExternalInput")

    with tile.TileContext(nc) as tc:
        with (
            tc.tile_pool(name="gp", bufs=1) as gpool,
            tc.tile_pool(name="mp", bufs=1) as mpool,
            tc.tile_pool(name="ob", bufs=1) as opool,
            tc.tile_pool(name="mn", bufs=1) as mnpool,
            tc.tile_pool(name="dv", bufs=4) as dvpool,
            tc.tile_pool(name="ps", bufs=4, space="PSUM") as pspool,
        ):
          for rep in range(repeat):
            # --- input loads, issued in consumption order ---------------
            # G chunk 0 is split into 2-k-tile pieces so the first matmul
            # waits for ~100KB, not the whole chunk.
            def g_load(ci, split):
                n0, w = chunks[ci]
                gt = gpool.tile([P, KT_N * w], mm_dt, tag=f"g{ci}",
                                name=f"g_{rep}_{ci}")
                step = KT_N // split
                for s in range(split):
                    f0, f1 = s * step * w, (s + 1) * step * w
                    nc.sync.dma_start(out=gt[:, f0:f1],
                                      in_=gcs[ci][:, f0:f1])
                return gt

            def m_load(b):
                mt = mpool.tile([P, KT_N * P], mm_dt, tag=f"m{b}",
                                name=f"m_{rep}_{b}")
                nc.sync.dma_start(out=mt[:], in_=mb[b, :, :])
                return mt

            g_sb = [None] * nch
            n0, w0 = chunks[0]
            g_sb[0] = gpool.tile([P, KT_N * w0], mm_dt, tag="g0",
                                 name=f"g_{rep}_0")
            # first 2 k-tiles of g0, then m block 0, then the rest of g0
            nc.sync.dma_start(out=g_sb[0][:, :2 * w0], in_=gcs[0][:, :2 * w0])
            m_sb = [m_load(0)]
            for s in range(1, 5):
                f0, f1 = s * 2 * w0, (s + 1) * 2 * w0
                nc.sync.dma_start(out=g_sb[0][:, f0:f1], in_=gcs[0][:, f0:f1])
            for b in range(1, nb):
                m_sb.append(m_load(b))
            for ci in range(1, nch):
                g_sb[ci] = g_load(ci, 2)
            mns = []
            if with_identity:
                for b in range(nb):
                    mn = mnpool.tile([P, k_msg], mm_dt, tag=f"mn{b}", name=f"mn_{rep}_{b}")
                    nc.sync.dma_start(out=mn[:], in_=mnat[b * P:(b + 1) * P, :])
                    mns.append(mn)

            # --- compute: chunk-outer, block, k-tile --------------------
            obt = [opool.tile([P, n_bits], out_dt, tag=f"ob{b}", name=f"ob_{rep}_{b}")
                   for b in range(nb)]
            for ci, (n0, w) in enumerate(chunks):
                for b in range(nb):
                    ps = pspool.tile([P, 512], f32, tag="ps",
                                     name=f"ps_{rep}_{ci}_{b}")
                    for t in range(KT_N):
                        nc.tensor.matmul(
                            ps[:, :w],
                            m_sb[b][:, t * P:(t + 1) * P],
                            g_sb[ci][:, t * w:(t + 1) * w],
                            start=(t == 0),
                            stop=(t == KT_N - 1),
                        )
                    # parity -> BPSK: p = int(d) & 1 ; out = -2p + 1
                    it = dvpool.tile([P, 512], i32, tag="pi",
                                     name=f"pi_{rep}_{ci}_{b}")
                    nc.vector.tensor_copy(it[:, :w], ps[:, :w])
                    pt = dvpool.tile([P, 512], i32, tag="pp",
                                     name=f"pp_{rep}_{ci}_{b}")
                    nc.vector.tensor_scalar(
                        pt[:, :w], it[:, :w], 1, None, op0=Alu.bitwise_and,
                    )
                    nc.vector.tensor_scalar(
                        obt[b][:, base_col + n0:base_col + n0 + w],
                        pt[:, :w], -2.0, 1.0, op0=Alu.mult, op1=Alu.add,
                    )
                    if with_identity and ci == 1:
                        # identity half: mnat DMA has landed by pass 1
                        nc.vector.tensor_scalar(
                            obt[b][:, 0:k_msg], mns[b][:], -2.0, 1.0,
                            op0=Alu.mult, op1=Alu.add,
                        )
                    # stream the output out in two pieces: everything
                    # finished after the second-to-last pass, remainder
                    # after the last pass (shrinks the kernel tail).
                    split_col = base_col + chunks[-1][0]
                    if ci == nch - 2:
                        nc.sync.dma_start(
                            out=out[b * P:(b + 1) * P, 0:split_col],
                            in_=obt[b][:, 0:split_col])
                    elif ci == nch - 1:
                        nc.sync.dma_start(
                            out=out[b * P:(b + 1) * P, split_col:n_bits],
                            in_=obt[b][:, split_col:n_bits])

    nc.compile()
    return nc


def _get_nc(fast: bool, repeat: int = 1):
    key = ("fast" if fast else "full", USE_FP8, repeat)
    if key not in _CACHE:
        if fast:
            _CACHE[key] = _build(B_LOC, K_MSG, N_BITS - K_MSG, N_BITS,
                                 K_MSG, True, repeat=repeat)
        else:
            _CACHE[key] = _build(B_LOC, K_MSG, N_BITS, N_BITS, 0, False,
                                 repeat=repeat)
    return _CACHE[key]


def _tile_kmaj(arr_kmaj, w):
    """[K_PAD, w]-sliced column-major-by-k-tile host retiling:
    returns [128, KT_N * w] with free index f = t*w + j -> arr[t*128+p, j]."""
    return np.ascontiguousarray(
        arr_kmaj.reshape(KT_N, P, w).transpose(1, 0, 2).reshape(P, KT_N * w))


def _prep_inputs(m, G, fast: bool):
    """Host-side marshaling: casts, transposes, padding, tiling."""
    mm_dt = _mm_np_dtype()
    nb = B_LOC // P
    if fast:
        g_rows = G[K_MSG:N_BITS]  # parity rows only
    else:
        g_rows = G
    n_par = g_rows.shape[0]
    gT = np.zeros((K_PAD, n_par), dtype=mm_dt)
    gT[:K_MSG] = g_rows.T.astype(mm_dt)
    g_maps = {f"gc{ci}": _tile_kmaj(gT[:, n0:n0 + w], w)
              for ci, (n0, w) in enumerate(_chunks(n_par))}

    in_maps = []
    for c in range(N_CORES):
        m_c = m[c * B_LOC:(c + 1) * B_LOC].astype(mm_dt)
        mTp = np.zeros((K_PAD, B_LOC), dtype=mm_dt)
        mTp[:K_MSG] = m_c.T
        # mb[b, p, t*128 + cc] = m_c[b*128 + cc, t*128 + p]
        mbt = np.stack([_tile_kmaj(mTp[:, b * P:(b + 1) * P], P)
                        for b in range(nb)])
        im = {"mb": mbt, **g_maps}
        if fast:
            im["mnat"] = np.ascontiguousarray(m_c)
        in_maps.append(im)
    return in_maps


def _run(m, G, trace=False):
    from concourse.bass_utils import run_bass_kernel_spmd

    fast = bool(
        np.array_equal(G[:K_MSG], np.eye(K_MSG, dtype=G.dtype))
        and ((G == 0) | (G == 1)).all()
    )
    nc = _get_nc(fast)
    in_maps = _prep_inputs(m, G, fast)
    res = run_bass_kernel_spmd(
        nc, in_maps, core_ids=list(range(N_CORES)), trace=trace,
    )
    parts = [res.results[c]["out"] for c in range(N_CORES)]
    full = np.concatenate(parts, axis=0).astype(np.float32)
    return full, res


def kernel(m, G, snr=None):
    m = np.asarray(m)
    G = np.asarray(G)
    full, _ = _run(m, G, trace=False)
    return full
